# revision 21
# baseline (speedup 1.0000x reference)
"""Trainium2 Bass kernel for nn_DynamicHybridModulation.

Sharding: data-parallel over batch (B=8 -> 8 cores, one batch each).  The
only cross-core communication is a 6-float AllReduce for the global
BatchNorm statistics of the bias branch.

v2 restructure (from trace analysis of the fp32r baseline):
- Score/context/value matmuls run in fp16 (1 cyc/row on the PE instead of
  2+ for fp32r; q is exactly ternary in f16, k rounds at ~5e-4 which is
  noise against the spike-threshold error already present).
- Phase B parks the raw scores in SBUF as f16 (preps) while computing the
  t-stats; phase C consumes them directly, so scores are computed exactly
  once and all phase-C elementwise runs at DVE 2x/4x f16 modes from SBUF
  instead of 1x from PSUM.
- The t-statistics are plain counts everywhere ((s>=16) via DVE
  tensor_scalar with accum_out for the row stat; a ones-matmul per kt
  accumulated on the PE for the column stat), dropping the sign-sum
  affine-fixup bookkeeping of the baseline.
- A warmup AllReduce on junk data runs under phase A so the real 24-byte
  stats AllReduce does not pay the collective lib-load/rendezvous latency
  at the BN barrier.
- exp is batched per head ([128,2048] in one ACT op), the softmax
  denominators are gathered to a [12,512] tile and inverted in one DVE op,
  and the output is written f16 (host casts to f32).
- Math bookkeeping (as baseline): the kernel computes S'' = 16*scores_ref
  via ternary q'' = -spike(q_lin) and k'' = -(spike(k_lin) + k_lin); the
  /16 is folded into the t-threshold (>=16), the exp scale (1/16) and the
  sh gate (sh16 = 16*K_BIAS*sh).  Q/K/V biases ride on an augmented
  contraction row (row 768 of the padded weights = bias, ones row in hs^T);
  that row is skipped when all biases are zero (this problem).
"""

import numpy as np

try:
    import concourse  # noqa: F401
except ImportError:  # pragma: no cover
    import sys

    for p in ("/opt/trn_rl_repo", "/root/.axon_site/_ro/trn_rl_repo"):
        sys.path.insert(0, p)

import concourse.bass as bass  # noqa: E402,F401
import concourse.tile as tile  # noqa: E402
from concourse import bacc, mybir  # noqa: E402
from concourse.bass_utils import run_bass_kernel_spmd  # noqa: E402

F32 = mybir.dt.float32
F32R = mybir.dt.float32r
BF16 = mybir.dt.bfloat16
F16 = mybir.dt.float16
ALU = mybir.AluOpType
ACTF = mybir.ActivationFunctionType

B, S, DM, H, D, R = 8, 512, 768, 12, 64, 3
NT = DM // 128  # 6 dout tiles
KT = S // 128  # 4 s tiles
NI = NT + 1  # 7 contraction tiles (6 x 128 + bias row block)
N_TOT = float(B * 2 * S)

_CACHE = {}


def _round_fp32r(x):
    """Round fp32 to the 11-explicit-mantissa-bit grid the PE uses for
    float32r operands (calibrated against hardware)."""
    u = np.ascontiguousarray(x, np.float32).view(np.uint32).copy()
    u = (u + np.uint32(0x800)) & np.uint32(0xFFFFF000)
    return u.view(np.float32)


def _build(mask_nonzero, bias_nonzero):
    nc = bacc.Bacc("TRN2", target_bir_lowering=False, debug=False, num_devices=8)

    hsT_d = nc.dram_tensor("hsT", [128, NI, S], F32R, kind="ExternalInput").ap()
    hsT16_d = nc.dram_tensor("hsT16", [128, NI, S], F16, kind="ExternalInput").ap()
    wq_d = nc.dram_tensor("wq", [NT, 128, NI, 128], F32R, kind="ExternalInput").ap()
    wk_d = nc.dram_tensor("wk", [NT, 128, NI, 128], F32R, kind="ExternalInput").ap()
    wv16_d = nc.dram_tensor("wv16", [128, NI, DM], F16, kind="ExternalInput").ap()
    mask_d = (
        nc.dram_tensor("mask_cols", [128, KT], F32, kind="ExternalInput").ap()
        if mask_nonzero
        else None
    )
    conv1T_d = nc.dram_tensor("conv1T", [H, R], F16, kind="ExternalInput").ap()
    sel_d = nc.dram_tensor("sel", [H, H * 128], F16, kind="ExternalInput").ap()
    convhT_d = nc.dram_tensor("convhT", [R, H], F16, kind="ExternalInput").ap()
    convwT_d = nc.dram_tensor("convwT", [R, H], F16, kind="ExternalInput").ap()
    gamma_d = nc.dram_tensor("gamma", [R, 1], F32, kind="ExternalInput").ap()
    beta_d = nc.dram_tensor("beta", [R, 1], F32, kind="ExternalInput").ap()
    outT_d = nc.dram_tensor("outT", [DM, S], F16, kind="ExternalOutput").ap()
    ar_in_d = nc.dram_tensor("ar_bounce", [R, 2], F32).ap()
    ar_out_d = nc.dram_tensor("ar_shared", [R, 2], F32, addr_space="Shared").ap()
    war_in_d = nc.dram_tensor("war_bounce", [R, 2], F32).ap()
    war_out_d = nc.dram_tensor("war_shared", [R, 2], F32, addr_space="Shared").ap()

    ni = NI if bias_nonzero else NT  # skip the bias row pass when all-zero
    with tile.TileContext(nc) as tc:
        with (
            tc.tile_pool(name="const", bufs=1) as cpool,
            tc.tile_pool(name="wstream", bufs=3) as wpool,
            tc.tile_pool(name="big", bufs=1) as bigpool,
            tc.tile_pool(name="ctxs", bufs=12) as ctxpool,
            tc.tile_pool(name="sgn", bufs=6) as sgnpool,
            tc.tile_pool(name="shbp", bufs=3) as shbpool,
            tc.tile_pool(name="prep", bufs=3) as prepool,
            tc.tile_pool(name="ebuf", bufs=3) as epool,
            tc.tile_pool(name="wk2", bufs=4) as wk2pool,
            tc.tile_pool(name="wk3", bufs=6) as wk3pool,
            tc.tile_pool(name="p2k", bufs=3, space="PSUM") as p2kpool,
            tc.tile_pool(name="ps1", bufs=2, space="PSUM") as ps1pool,
        ):
            # ---- resident loads (partition-major, contiguous per line) ----
            hsT_t = cpool.tile([128, NI, S], F32R)
            for i in range(NI):
                nc.sync.dma_start(hsT_t[:, i, :], hsT_d[:, i, :])
            conv1T_t = cpool.tile([H, R], F16)
            nc.sync.dma_start(conv1T_t[:], conv1T_d[:])
            convhT_t = cpool.tile([R, H], F16)
            nc.sync.dma_start(convhT_t[:], convhT_d[:])
            convwT_t = cpool.tile([R, H], F16)
            nc.sync.dma_start(convwT_t[:], convwT_d[:])
            gamma_t = cpool.tile([R, 1], F32)
            nc.sync.dma_start(gamma_t[:], gamma_d[:])
            beta_t = cpool.tile([R, 1], F32)
            nc.sync.dma_start(beta_t[:], beta_d[:])
            if mask_nonzero:
                mask_t = cpool.tile([128, KT], F32)
                nc.sync.dma_start(mask_t[:], mask_d[:])

            ones_f16 = cpool.tile([128, 1], F16)
            nc.gpsimd.memset(ones_f16[:], 1.0)
            # one-hot selector rows: sel_t[c, h*128+p] = (c == h); used as
            # matmul weights to broadcast row h of a [12, S] tile across
            # partitions on the PE (gpsimd partition_broadcast needs the
            # source in partition 0, which would cost a staging DMA per head)
            sel_t = cpool.tile([H, H * 128], F16)
            nc.sync.dma_start(sel_t[:], sel_d[:])

            # warmup AllReduce: pays the collective lib-load/rendezvous cost
            # under phase A so the real stats AllReduce is fast
            nc.gpsimd.collective_compute(
                "AllReduce",
                ALU.add,
                replica_groups=[list(range(8))],
                ins=[war_in_d[:]],
                outs=[war_out_d[:]],
            )

            # ---- persistent intermediates ----
            hsT16_t = cpool.tile([128, NI, S], F16)
            wv16_t = cpool.tile([128, NI, DM], F16)
            qT_t = bigpool.tile([128, NT, S], F16)  # -spike(q_lin)^T
            kT_t = bigpool.tile([128, NT, S], F16)  # -(spike+lin)(k_lin)^T
            v_t = bigpool.tile([128, KT, H * 65], F16)  # v with ones cols
            preps_t = bigpool.tile([128, H, KT * S], F16)  # parked scores
            cat_t = bigpool.tile([H, 2 * S], F16)  # count stats [xh | xw]
            xw_cols = bigpool.tile([128, H * KT], F16)  # count accum slots
            sw_cols = bigpool.tile([128, KT, H], F32)  # sigmoid(convw)
            sh_t = bigpool.tile([H, S], F16)  # 16*sigmoid(convh)
            den_t = bigpool.tile([H, S], F16)  # softmax denominators
            rec_t = bigpool.tile([H, S], F16)
            den32_t = bigpool.tile([H, S], F32)
            rec32_t = bigpool.tile([H, S], F32)

            for st in range(KT):
                nc.gpsimd.memset(
                    v_t[:, st, :].rearrange("p (h c) -> p h c", c=65)[:, :, 64:65], 1.0
                )

            # =========== PHASE A: q/k projections (fp32r) ===========
            for j in range(NT):
                for proj, w_d, dst in (("q", wq_d, qT_t), ("k", wk_d, kT_t)):
                    w_t = wpool.tile([128, NI, 128], F32R, tag="wblk")
                    for c in range(4):
                        nc.sync.dma_start(
                            w_t[c * 32 : (c + 1) * 32, :, :],
                            w_d[j][c * 32 : (c + 1) * 32],
                        )
                    pj = p2kpool.tile([128, 1024], F32, tag="ps")
                    pa = pj[:, :512]
                    for i in range(ni):
                        nc.tensor.matmul(
                            pa,
                            w_t[:, i, :],
                            hsT_t[:, i, :],
                            start=(i == 0),
                            stop=(i == ni - 1),
                        )
                    if proj == "q":
                        # q'' = (x<=-1) - (x>=1) = -spike(x)
                        t1 = wk3pool.tile([128, S], F16, tag="qk_tmp")
                        nc.vector.tensor_scalar(t1[:], pa, 1.0, None, ALU.is_ge)
                        nc.vector.scalar_tensor_tensor(
                            dst[:, j, :], pa, -1.0, t1[:], ALU.is_le, ALU.subtract
                        )
                    else:
                        # k'' = (x<=-1) - (x>=1) - x = -(spike(x) + x)
                        t1 = wk3pool.tile([128, S], F16, tag="qk_tmp")
                        nc.vector.tensor_scalar(
                            t1[:], pa, 1.0, -1.0, ALU.is_ge, ALU.mult
                        )
                        t2 = wk3pool.tile([128, S], F16, tag="qk_tmp2")
                        nc.vector.scalar_tensor_tensor(
                            t2[:], pa, -1.0, t1[:], ALU.is_le, ALU.add
                        )
                        nc.vector.tensor_tensor(dst[:, j, :], t2[:], pa, ALU.subtract)

            # v-projection operands stream in during phase B (HBM is idle then)
            for i in range(NI):
                nc.sync.dma_start(hsT16_t[:, i, :], hsT16_d[:, i, :])
            for c in range(4):
                nc.sync.dma_start(
                    wv16_t[c * 32 : (c + 1) * 32, :, :], wv16_d[c * 32 : (c + 1) * 32]
                )

            # =========== PHASE B: scores -> park f16 + count stats ===========
            with nc.allow_low_precision(reason="counts are small ints, f16 exact"):
                for h in range(H):
                    jh, p0 = divmod(h * D, 128)
                    # scores in kt pairs; one wide PSUM->SBUF f16 drain each
                    for kt2 in range(2):
                        psb = p2kpool.tile([128, 1024], F32, tag="ps")
                        for u in range(2):
                            kt = 2 * kt2 + u
                            nc.tensor.matmul(
                                psb[:, u * 512 : (u + 1) * 512],
                                kT_t[p0 : p0 + D, jh, kt * 128 : (kt + 1) * 128],
                                qT_t[p0 : p0 + D, jh, :],
                                start=True,
                                stop=True,
                            )
                        dst = preps_t[:, h, kt2 * 1024 : (kt2 + 1) * 1024]
                        if (h + kt2) % 3 == 0:
                            nc.vector.tensor_copy(dst, psb[:])
                        else:
                            nc.scalar.copy(dst, psb[:])
                    # t counts: row stat via accum_out, col stat via ones-MM
                    pxh = ps1pool.tile([65, S], F32, tag="pacc")
                    for kt in range(KT):
                        sg = sgnpool.tile([128, S], F16, tag="sgn")
                        nc.vector.tensor_scalar(
                            sg[:],
                            preps_t[:, h, kt * S : (kt + 1) * S],
                            16.0,
                            None,
                            ALU.is_ge,
                            ALU.add,
                            accum_out=xw_cols[:, h * KT + kt : h * KT + kt + 1],
                        )
                        nc.tensor.matmul(
                            pxh[0:1, :],
                            ones_f16[:],
                            sg[:],
                            start=(kt == 0),
                            stop=(kt == KT - 1),
                        )
                    xh_row = wk2pool.tile([1, S], F16, tag="xhrow")
                    if h % 2 == 0:
                        nc.scalar.copy(xh_row[:], pxh[0:1, :])
                    else:
                        nc.vector.tensor_copy(xh_row[:], pxh[0:1, :])
                    nc.sync.dma_start(cat_t[h : h + 1, 0:S], xh_row[:])
                    for kt in range(KT):
                        nc.sync.dma_start(
                            cat_t[h : h + 1, S + kt * 128 : S + (kt + 1) * 128],
                            xw_cols[:, h * KT + kt : h * KT + kt + 1],
                        )

            # =========== MID: conv1 -> stats -> AllReduce; v-proj cover ======
            pyh = p2kpool.tile([128, 1024], F32, tag="ps")
            nc.tensor.matmul(
                pyh[0:R, 0:S], conv1T_t[:], cat_t[:, 0:S], start=True, stop=True
            )
            nc.tensor.matmul(
                pyh[0:R, 512:1024], conv1T_t[:], cat_t[:, S:], start=True, stop=True
            )
            y_t = bigpool.tile([R, 2 * S], F32)
            nc.scalar.copy(y_t[:], pyh[0:R, :])

            stats_t = bigpool.tile([R, 2], F32)
            nc.vector.tensor_reduce(
                stats_t[:, 0:1], y_t[:], mybir.AxisListType.X, ALU.add
            )
            yn_t = bigpool.tile([R, 2 * S], F32)
            nc.vector.tensor_tensor(yn_t[:], y_t[:], y_t[:], ALU.mult)
            nc.vector.tensor_reduce(
                stats_t[:, 1:2], yn_t[:], mybir.AxisListType.X, ALU.add
            )
            nc.sync.dma_start(ar_in_d[:], stats_t[:])
            nc.gpsimd.collective_compute(
                "AllReduce",
                ALU.add,
                replica_groups=[list(range(8))],
                ins=[ar_in_d[:]],
                outs=[ar_out_d[:]],
            )

            # value projection in f16 fills the collective window
            for st in range(KT):
                for dh in range(2):
                    pv = p2kpool.tile([128, 1024], F32, tag="ps")
                    for i in range(ni):
                        nc.tensor.matmul(
                            pv[:, :384],
                            hsT16_t[:, i, st * 128 : (st + 1) * 128],
                            wv16_t[:, i, dh * 384 : (dh + 1) * 384],
                            start=(i == 0),
                            stop=(i == ni - 1),
                        )
                    dst = v_t[:, st, dh * 6 * 65 : (dh + 1) * 6 * 65].rearrange(
                        "p (h c) -> p h c", c=65
                    )[:, :, 0:64]
                    src = pv[:, :384].rearrange("p (h c) -> p h c", c=64)
                    if dh == 0:
                        nc.scalar.copy(dst, src)
                    else:
                        nc.vector.tensor_copy(dst, src)

            gstats_t = bigpool.tile([R, 2], F32)
            nc.sync.dma_start(gstats_t[:], ar_out_d[:])

            # BN coefficients from global stats
            mom_t = bigpool.tile([R, 2], F32)
            nc.vector.tensor_scalar(mom_t[:], gstats_t[:], 1.0 / N_TOT, None, ALU.mult)
            mu_t = mom_t[:, 0:1]
            ex2_t = mom_t[:, 1:2]
            nvar_t = bigpool.tile([R, 1], F32)
            nc.vector.scalar_tensor_tensor(
                nvar_t[:], mu_t[:], mu_t[:], ex2_t[:], ALU.mult, ALU.subtract
            )
            vpe_t = bigpool.tile([R, 1], F32)
            nc.vector.tensor_scalar(vpe_t[:], nvar_t[:], -1.0, 1e-5, ALU.mult, ALU.add)
            sd_t = bigpool.tile([R, 1], F32)
            nc.scalar.sqrt(sd_t[:], vpe_t[:])
            inv_t = bigpool.tile([R, 1], F32)
            nc.vector.reciprocal(inv_t[:], sd_t[:])
            gp_t = bigpool.tile([R, 1], F32)
            nc.vector.tensor_tensor(gp_t[:], gamma_t[:], inv_t[:], ALU.mult)
            mg_t = bigpool.tile([R, 1], F32)
            nc.vector.tensor_tensor(mg_t[:], mu_t[:], gp_t[:], ALU.mult)
            bp_t = bigpool.tile([R, 1], F32)
            nc.vector.tensor_tensor(bp_t[:], beta_t[:], mg_t[:], ALU.subtract)
            nc.vector.tensor_scalar(
                yn_t[:], y_t[:], gp_t[:], bp_t[:], ALU.mult, ALU.add
            )
            yr_t = bigpool.tile([R, 2 * S], F16)
            nc.scalar.activation(yr_t[:], yn_t[:], ACTF.Relu)

            psh = ps1pool.tile([65, S], F32, tag="pacc")
            nc.tensor.matmul(psh[0:H, :], convhT_t[:], yr_t[:, :S], start=True, stop=True)
            sh_sig = bigpool.tile([H, S], F16)
            nc.scalar.activation(sh_sig[:], psh[0:H, :], ACTF.Sigmoid)
            nc.vector.tensor_scalar(sh_t[:], sh_sig[:], 16.0, None, ALU.mult)
            for st in range(KT):
                psw = p2kpool.tile([128, 1024], F32, tag="ps")
                nc.tensor.matmul(
                    psw[:, 0:H],
                    yr_t[:, S + st * 128 : S + (st + 1) * 128],
                    convwT_t[:],
                    start=True,
                    stop=True,
                )
                nc.scalar.activation(sw_cols[:, st, :], psw[:, 0:H], ACTF.Sigmoid)

            # =========== PHASE C: gates + softmax + context ===========
            NB = 1  # output flush batches (DVE base partition must be aligned)
            HB = H // NB
            ctx_stages = []
            for h in range(H):
                # broadcast sh row h across partitions with a one-hot matmul
                pshb = p2kpool.tile([128, 1024], F32, tag="ps")
                nc.tensor.matmul(
                    pshb[:, 0:S],
                    sel_t[:, h * 128 : (h + 1) * 128],
                    sh_t[:],
                    start=True,
                    stop=True,
                )
                shb = shbpool.tile([128, S], F16, tag="shb")
                if h % 2 == 0:
                    nc.scalar.copy(shb[:], pshb[:, 0:S])
                else:
                    nc.vector.tensor_copy(shb[:], pshb[:, 0:S])
                pre_t = prepool.tile([128, KT, S], F16, tag="pre")
                for kt in range(KT):
                    prs = preps_t[:, h, kt * S : (kt + 1) * S]
                    g1 = wk3pool.tile([128, S], F16, tag="g1")
                    nc.vector.scalar_tensor_tensor(
                        g1[:], prs, 16.0, shb[:], ALU.is_ge, ALU.mult
                    )
                    nc.vector.scalar_tensor_tensor(
                        pre_t[:, kt, :],
                        g1[:],
                        sw_cols[:, kt, h : h + 1],
                        prs,
                        ALU.mult,
                        ALU.add,
                    )
                    if mask_nonzero:
                        nc.vector.tensor_scalar(
                            pre_t[:, kt, :],
                            pre_t[:, kt, :],
                            mask_t[:, kt : kt + 1],
                            None,
                            ALU.add,
                        )
                e_t = epool.tile([128, KT, S], F16, tag="ebuf")
                nc.scalar.activation(e_t[:], pre_t[:], ACTF.Exp, scale=1.0 / 16.0)
                # keepalive: tiny junk MM keyed on this head's gates keeps the
                # PE HAM window warm through the elementwise-heavy stretch
                pjunk = p2kpool.tile([128, 1024], F32, tag="ps")
                nc.tensor.matmul(
                    pjunk[0:1, 0:S], ones_f16[:], pre_t[:, 0, :], start=True, stop=True
                )
                pctx = ps1pool.tile([65, S], F32, tag="pacc")
                for kt in range(KT):
                    nc.tensor.matmul(
                        pctx[:],
                        v_t[:, kt, h * 65 : (h + 1) * 65],
                        e_t[:, kt, :],
                        start=(kt == 0),
                        stop=(kt == KT - 1),
                    )
                ctx_stage = ctxpool.tile([D + 1, S], F16, tag="ctxs")
                if h % 2 == 0:
                    nc.scalar.copy(ctx_stage[:], pctx[:])
                else:
                    nc.vector.tensor_copy(ctx_stage[:], pctx[:])
                ctx_stages.append(ctx_stage)
                nc.gpsimd.dma_start(
                    den_t[h : h + 1, :], ctx_stage[D : D + 1, :]
                )
                half, hh = divmod(h, HB)
                if hh == HB - 1:
                    hs0 = half * HB
                    hsl = slice(hs0, hs0 + HB)
                    nc.vector.tensor_copy(den32_t[hsl, :], den_t[hsl, :])
                    nc.vector.reciprocal_approx_fast(rec32_t[hsl, :], den32_t[hsl, :])
                    nc.vector.tensor_copy(rec_t[hsl, :], rec32_t[hsl, :])
                    for h2 in range(hs0, hs0 + HB):
                        # broadcast the reciprocal row to D partitions (PE)
                        prb = p2kpool.tile([128, 1024], F32, tag="ps")
                        nc.tensor.matmul(
                            prb[0:D, 0:S],
                            sel_t[:, h2 * 128 : h2 * 128 + D],
                            rec_t[:],
                            start=True,
                            stop=True,
                        )
                        outp = wk3pool.tile([D, S], F16, tag="outp")
                        nc.vector.tensor_tensor(
                            outp[:], ctx_stages[h2][0:D, :], prb[0:D, 0:S], ALU.mult
                        )
                        nc.scalar.dma_start(outT_d[h2 * D : (h2 + 1) * D, :], outp[:])

    nc.compile()
    return nc


def _prep_inputs(
    hidden_states,
    attention_mask,
    Wq,
    bq,
    Wk,
    bk,
    Wv,
    bv,
    conv1_w,
    bn_gamma,
    bn_beta,
    convh_w,
    convw_w,
):
    f32 = np.float32
    f16 = np.float16

    def pad_w(W, b):
        Wp = np.zeros((NI * 128, DM), f32)
        Wp[:DM] = np.asarray(W, f32)
        Wp[DM] = np.asarray(b, f32)
        return Wp

    def col_blocks(Wp):
        # [6(j), 128(p), 7(i), 128(c)]: per-partition contiguous DMA lines
        return np.ascontiguousarray(
            _round_fp32r(Wp).reshape(NI, 128, NT, 128).transpose(2, 1, 0, 3)
        )

    wq_p = col_blocks(pad_w(Wq, bq))
    wk_p = col_blocks(pad_w(Wk, bk))
    wv16_p = np.ascontiguousarray(
        pad_w(Wv, bv).reshape(NI, 128, DM).transpose(1, 0, 2).astype(f16)
    )
    conv1 = np.asarray(conv1_w, f32)
    # counts are scaled to means by folding 1/S into conv1
    conv1T = np.ascontiguousarray((conv1.T / float(S)).astype(f16))
    sel = np.zeros((H, H * 128), f16)
    for h in range(H):
        sel[h, h * 128 : (h + 1) * 128] = 1.0
    convhT = np.ascontiguousarray(np.asarray(convh_w, f32).T.astype(f16))
    convwT = np.ascontiguousarray(np.asarray(convw_w, f32).T.astype(f16))
    gamma = np.asarray(bn_gamma, f32).reshape(R, 1)
    beta = np.asarray(bn_beta, f32).reshape(R, 1)

    hs = np.asarray(hidden_states, f32)
    am = np.asarray(attention_mask, f32)
    in_maps = []
    for b in range(B):
        hsT = np.zeros((NI * 128, S), f32)
        hsT[:DM] = _round_fp32r(hs[b].T)
        hsT[DM] = 1.0
        hsT_p = np.ascontiguousarray(hsT.reshape(NI, 128, S).transpose(1, 0, 2))
        hsT16_p = hsT_p.astype(f16)
        extra = {}
        if np.any(am):
            # 16*mask as per-partition columns [128(k%128), KT(k//128)]
            mc = (16.0 * am[b, 0, 0]).reshape(KT, 128).T
            extra["mask_cols"] = np.ascontiguousarray(mc.astype(f32))
        in_maps.append(
            dict(
                hsT=hsT_p,
                hsT16=hsT16_p,
                wq=wq_p,
                wk=wk_p,
                wv16=wv16_p,
                **extra,
                conv1T=conv1T,
                sel=sel,
                convhT=convhT,
                convwT=convwT,
                gamma=gamma,
                beta=beta,
            )
        )
    return in_maps


def _run(inputs, trace=False, trace_kwargs=None):
    mask_nonzero = bool(np.any(np.asarray(inputs["attention_mask"])))
    bias_nonzero = any(
        bool(np.any(np.asarray(inputs[k]))) for k in ("bq", "bk", "bv")
    )
    key = ("nc", mask_nonzero, bias_nonzero)
    if key not in _CACHE:
        _CACHE[key] = _build(mask_nonzero, bias_nonzero)
    nc = _CACHE[key]
    in_maps = _prep_inputs(**inputs)
    res = run_bass_kernel_spmd(
        nc, in_maps, list(range(8)), trace=trace, **(trace_kwargs or {})
    )
    out = np.stack(
        [np.ascontiguousarray(r["outT"].T).astype(np.float32) for r in res.results]
    )
    return out, res


def kernel(**inputs):
    out, _ = _run(inputs, trace=False)
    return out


# revision 32
# speedup vs baseline: 1.1848x; 1.1848x over previous
"""Trainium2 Bass kernel for nn_DynamicHybridModulation.

Sharding: data-parallel over batch (B=8 -> 8 cores, one batch each).  The
only cross-core communication is a 6-float AllReduce for the global
BatchNorm statistics of the bias branch.

v2 restructure (from trace analysis of the fp32r baseline):
- Score/context/value matmuls run in fp16 (1 cyc/row on the PE instead of
  2+ for fp32r; q is exactly ternary in f16, k rounds at ~5e-4 which is
  noise against the spike-threshold error already present).
- Phase B parks the raw scores in SBUF as f16 (preps) while computing the
  t-stats; phase C consumes them directly, so scores are computed exactly
  once and all phase-C elementwise runs at DVE 2x/4x f16 modes from SBUF
  instead of 1x from PSUM.
- The t-statistics are plain counts everywhere ((s>=16) via DVE
  tensor_scalar with accum_out for the row stat; a ones-matmul per kt
  accumulated on the PE for the column stat), dropping the sign-sum
  affine-fixup bookkeeping of the baseline.
- A warmup AllReduce on junk data runs under phase A so the real 24-byte
  stats AllReduce does not pay the collective lib-load/rendezvous latency
  at the BN barrier.
- exp is batched per head ([128,2048] in one ACT op), the softmax
  denominators are gathered to a [12,512] tile and inverted in one DVE op,
  and the output is written f16 (host casts to f32).
- Math bookkeeping (as baseline): the kernel computes S'' = 16*scores_ref
  via ternary q'' = -spike(q_lin) and k'' = -(spike(k_lin) + k_lin); the
  /16 is folded into the t-threshold (>=16), the exp scale (1/16) and the
  sh gate (sh16 = 16*K_BIAS*sh).  Q/K/V biases ride on an augmented
  contraction row (row 768 of the padded weights = bias, ones row in hs^T);
  that row is skipped when all biases are zero (this problem).
"""

import numpy as np

try:
    import concourse  # noqa: F401
except ImportError:  # pragma: no cover
    import sys

    for p in ("/opt/trn_rl_repo", "/root/.axon_site/_ro/trn_rl_repo"):
        sys.path.insert(0, p)

import concourse.bass as bass  # noqa: E402,F401
import concourse.tile as tile  # noqa: E402
from concourse import bacc, mybir  # noqa: E402
from concourse.bass_utils import run_bass_kernel_spmd  # noqa: E402

F32 = mybir.dt.float32
F32R = mybir.dt.float32r
BF16 = mybir.dt.bfloat16
F16 = mybir.dt.float16
ALU = mybir.AluOpType
ACTF = mybir.ActivationFunctionType

B, S, DM, H, D, R = 8, 512, 768, 12, 64, 3
NT = DM // 128  # 6 dout tiles
KT = S // 128  # 4 s tiles
NI = NT + 1  # 7 contraction tiles (6 x 128 + bias row block)
N_TOT = float(B * 2 * S)

_CACHE = {}


def _round_fp32r(x):
    """Round fp32 to the 11-explicit-mantissa-bit grid the PE uses for
    float32r operands (calibrated against hardware)."""
    u = np.ascontiguousarray(x, np.float32).view(np.uint32).copy()
    u = (u + np.uint32(0x800)) & np.uint32(0xFFFFF000)
    return u.view(np.float32)


def _build(mask_nonzero, bias_nonzero):
    nc = bacc.Bacc("TRN2", target_bir_lowering=False, debug=False, num_devices=8)

    hsT_d = nc.dram_tensor("hsT", [128, NI, S], F32R, kind="ExternalInput").ap()
    hsT16_d = nc.dram_tensor("hsT16", [128, NI, S], F16, kind="ExternalInput").ap()
    wq_d = nc.dram_tensor("wq", [NT, 128, NI, 128], F32R, kind="ExternalInput").ap()
    wk_d = nc.dram_tensor("wk", [NT, 128, NI, 128], F32R, kind="ExternalInput").ap()
    wv16_d = nc.dram_tensor("wv16", [128, NI, DM], F16, kind="ExternalInput").ap()
    mask_d = (
        nc.dram_tensor("mask_cols", [128, KT], F32, kind="ExternalInput").ap()
        if mask_nonzero
        else None
    )
    conv1T_d = nc.dram_tensor("conv1T", [H, R], F16, kind="ExternalInput").ap()
    sel_d = nc.dram_tensor("sel", [H, H * 128], F16, kind="ExternalInput").ap()
    convhT_d = nc.dram_tensor("convhT", [R, H], F16, kind="ExternalInput").ap()
    convwT_d = nc.dram_tensor("convwT", [R, H], F16, kind="ExternalInput").ap()
    gamma_d = nc.dram_tensor("gamma", [R, 1], F32, kind="ExternalInput").ap()
    beta_d = nc.dram_tensor("beta", [R, 1], F32, kind="ExternalInput").ap()
    outT_d = nc.dram_tensor("outT", [DM, S], F16, kind="ExternalOutput").ap()
    ar_in_d = nc.dram_tensor("ar_bounce", [R, 2], F32).ap()
    ar_out_d = nc.dram_tensor("ar_shared", [R, 2], F32, addr_space="Shared").ap()

    ni = NI if bias_nonzero else NT  # skip the bias row pass when all-zero
    with tile.TileContext(nc) as tc:
        with (
            tc.tile_pool(name="const", bufs=1) as cpool,
            tc.tile_pool(name="wstream", bufs=3) as wpool,
            tc.tile_pool(name="big", bufs=1) as bigpool,
            tc.tile_pool(name="ctxs", bufs=12) as ctxpool,
            tc.tile_pool(name="shbp", bufs=2) as shbpool,
            tc.tile_pool(name="prep", bufs=2) as prepool,
            tc.tile_pool(name="ebuf", bufs=2) as epool,
            tc.tile_pool(name="wk3", bufs=2) as wk3pool,
            tc.tile_pool(name="sgp", bufs=2) as sgpool,
            tc.tile_pool(name="p2k", bufs=3, space="PSUM") as p2kpool,
            tc.tile_pool(name="ps1", bufs=2, space="PSUM") as ps1pool,
        ):
            # ---- resident loads (partition-major, contiguous per line) ----
            hsT_t = cpool.tile([128, NI, S], F32R)
            for i in range(NI):
                nc.sync.dma_start(hsT_t[:, i, :], hsT_d[:, i, :])
            conv1T_t = cpool.tile([H, R], F16)
            nc.sync.dma_start(conv1T_t[:], conv1T_d[:])
            convhT_t = cpool.tile([R, H], F16)
            nc.sync.dma_start(convhT_t[:], convhT_d[:])
            convwT_t = cpool.tile([R, H], F16)
            nc.sync.dma_start(convwT_t[:], convwT_d[:])
            gamma_t = cpool.tile([R, 1], F32)
            nc.sync.dma_start(gamma_t[:], gamma_d[:])
            beta_t = cpool.tile([R, 1], F32)
            nc.sync.dma_start(beta_t[:], beta_d[:])
            if mask_nonzero:
                mask_t = cpool.tile([128, KT], F32)
                nc.sync.dma_start(mask_t[:], mask_d[:])

            ones_f16 = cpool.tile([128, 1], F16)
            nc.gpsimd.memset(ones_f16[:], 1.0)
            # one-hot selector rows: sel_t[c, h*128+p] = (c == h); used as
            # matmul weights to broadcast row h of a [12, S] tile across
            # partitions on the PE (gpsimd partition_broadcast needs the
            # source in partition 0, which would cost a staging DMA per head)
            sel_t = cpool.tile([H, H * 128], F16)
            nc.sync.dma_start(sel_t[:], sel_d[:])

            # ---- persistent intermediates ----
            hsT16_t = cpool.tile([128, ni, S], F16)
            wv16_t = cpool.tile([128, ni, DM], F16)
            qT_t = bigpool.tile([128, NT, S], F16)  # -spike(q_lin)^T
            kT_t = bigpool.tile([128, NT, S], F16)  # -(spike+lin)(k_lin)^T
            v_t = bigpool.tile([128, KT, H * 65], F16)  # v with ones cols
            preps_t = bigpool.tile([128, H, KT * S], F16)  # parked scores
            cat_t = bigpool.tile([H, 2 * S], F16)  # count stats [xh | xw]
            xw_cols = bigpool.tile([128, H * KT], F16)  # count accum slots
            sw_cols = bigpool.tile([128, KT, H], F32)  # sigmoid(convw)
            sh_t = bigpool.tile([H, S], F16)  # 16*sigmoid(convh)
            den_t = bigpool.tile([H, S], F16)  # softmax denominators
            rec_t = bigpool.tile([H, S], F16)
            den32_t = bigpool.tile([H, S], F32)
            rec32_t = bigpool.tile([H, S], F32)

            for st in range(KT):
                nc.gpsimd.memset(
                    v_t[:, st, :].rearrange("p (h c) -> p h c", c=65)[:, :, 64:65], 1.0
                )

            # =========== PHASE A: q/k projections (fp32r) ===========
            for j in range(NT):
                for proj, w_d, dst in (("q", wq_d, qT_t), ("k", wk_d, kT_t)):
                    w_t = wpool.tile([128, NI, 128], F32R, tag="wblk")
                    for c in range(4):
                        nc.sync.dma_start(
                            w_t[c * 32 : (c + 1) * 32, :, :],
                            w_d[j][c * 32 : (c + 1) * 32],
                        )
                    pj = p2kpool.tile([128, 1024], F32, tag="ps")
                    pa = pj[:, :512]
                    for i in range(ni):
                        nc.tensor.matmul(
                            pa,
                            w_t[:, i, :],
                            hsT_t[:, i, :],
                            start=(i == 0),
                            stop=(i == ni - 1),
                        )
                    # threshold from a f16 copy: TS/TT then run at DVE 4x/2x
                    # modes instead of 1x from PSUM (flip-band widens by the
                    # f16 grid ~2.4e-4, noise vs the fp32r matmul error ~1e-3)
                    pa16 = wk3pool.tile([128, S], F16, tag="pa16")
                    nc.scalar.copy(pa16[:], pa)
                    t1 = wk3pool.tile([128, S], F16, tag="qk_tmp")
                    nc.vector.tensor_scalar(t1[:], pa16[:], 1.0, None, ALU.is_ge)
                    t2 = wk3pool.tile([128, S], F16, tag="qk_tmp2")
                    nc.vector.tensor_scalar(t2[:], pa16[:], -1.0, None, ALU.is_le)
                    if proj == "q":
                        # q'' = (x<=-1) - (x>=1) = -spike(x)
                        nc.vector.tensor_tensor(
                            dst[:, j, :], t2[:], t1[:], ALU.subtract
                        )
                    else:
                        # k'' = -spike(x) - x = -(spike(x) + x)
                        sp = wk3pool.tile([128, S], F16, tag="qk_tmp3")
                        nc.vector.tensor_tensor(sp[:], t2[:], t1[:], ALU.subtract)
                        nc.vector.tensor_tensor(
                            dst[:, j, :], sp[:], pa16[:], ALU.subtract
                        )

            # v-projection operands stream in during phase B (HBM is idle then)
            for i in range(ni):
                nc.sync.dma_start(hsT16_t[:, i, :], hsT16_d[:, i, :])
            for c in range(4):
                nc.sync.dma_start(
                    wv16_t[c * 32 : (c + 1) * 32, :, :],
                    wv16_d[c * 32 : (c + 1) * 32, 0:ni, :],
                )

            # =========== PHASE B: scores -> park f16 + count stats ===========
            with nc.allow_low_precision(reason="counts are small ints, f16 exact"):
                for h in range(H):
                    jh, p0 = divmod(h * D, 128)
                    # scores in kt pairs; one wide PSUM->SBUF f16 drain each
                    for kt2 in range(2):
                        psb = p2kpool.tile([128, 1024], F32, tag="ps")
                        for u in range(2):
                            kt = 2 * kt2 + u
                            nc.tensor.matmul(
                                psb[:, u * 512 : (u + 1) * 512],
                                kT_t[p0 : p0 + D, jh, kt * 128 : (kt + 1) * 128],
                                qT_t[p0 : p0 + D, jh, :],
                                start=True,
                                stop=True,
                            )
                        dst = preps_t[:, h, kt2 * 1024 : (kt2 + 1) * 1024]
                        if (h + kt2) % 3 == 0:
                            nc.vector.tensor_copy(dst, psb[:])
                        else:
                            nc.scalar.copy(dst, psb[:])
                    # t tile (parked for phase C); row stat via tensor_reduce,
                    # col stat via ones-MM accumulation on the PE
                    pxh = ps1pool.tile([65, S], F32, tag="pacc")
                    sg = sgpool.tile([128, KT * S], F16, tag="sg")
                    nc.vector.tensor_scalar(
                        sg[:], preps_t[:, h, :], 16.0, None, ALU.is_ge
                    )
                    nc.vector.tensor_reduce(
                        xw_cols[:, h * KT : (h + 1) * KT],
                        sg[:].rearrange("p (kt s) -> p kt s", s=S),
                        mybir.AxisListType.X,
                        ALU.add,
                    )
                    for kt in range(KT):
                        nc.tensor.matmul(
                            pxh[0:1, :],
                            ones_f16[:],
                            sg[:, kt * S : (kt + 1) * S],
                            start=(kt == 0),
                            stop=(kt == KT - 1),
                        )
                    xh_row = wk3pool.tile([1, S], F16, tag="xhrow")
                    if h % 2 == 0:
                        nc.scalar.copy(xh_row[:], pxh[0:1, :])
                    else:
                        nc.vector.tensor_copy(xh_row[:], pxh[0:1, :])
                    nc.sync.dma_start(cat_t[h : h + 1, 0:S], xh_row[:])
                    for kt in range(KT):
                        nc.sync.dma_start(
                            cat_t[h : h + 1, S + kt * 128 : S + (kt + 1) * 128],
                            xw_cols[:, h * KT + kt : h * KT + kt + 1],
                        )

            # =========== MID: conv1 -> stats -> AllReduce; v-proj cover ======
            pyh = p2kpool.tile([128, 1024], F32, tag="ps")
            nc.tensor.matmul(
                pyh[0:R, 0:S], conv1T_t[:], cat_t[:, 0:S], start=True, stop=True
            )
            nc.tensor.matmul(
                pyh[0:R, 512:1024], conv1T_t[:], cat_t[:, S:], start=True, stop=True
            )
            y_t = bigpool.tile([R, 2 * S], F32)
            nc.scalar.copy(y_t[:], pyh[0:R, :])

            stats_t = bigpool.tile([R, 2], F32)
            nc.vector.tensor_reduce(
                stats_t[:, 0:1], y_t[:], mybir.AxisListType.X, ALU.add
            )
            yn_t = bigpool.tile([R, 2 * S], F32)
            nc.vector.tensor_tensor(yn_t[:], y_t[:], y_t[:], ALU.mult)
            nc.vector.tensor_reduce(
                stats_t[:, 1:2], yn_t[:], mybir.AxisListType.X, ALU.add
            )
            nc.sync.dma_start(ar_in_d[:], stats_t[:])
            nc.gpsimd.collective_compute(
                "AllReduce",
                ALU.add,
                replica_groups=[list(range(8))],
                ins=[ar_in_d[:]],
                outs=[ar_out_d[:]],
            )

            # value projection in f16 fills the collective window
            for st in range(KT):
                for dh in range(2):
                    pv = p2kpool.tile([128, 1024], F32, tag="ps")
                    for i in range(ni):
                        nc.tensor.matmul(
                            pv[:, :384],
                            hsT16_t[:, i, st * 128 : (st + 1) * 128],
                            wv16_t[:, i, dh * 384 : (dh + 1) * 384],
                            start=(i == 0),
                            stop=(i == ni - 1),
                        )
                    dst = v_t[:, st, dh * 6 * 65 : (dh + 1) * 6 * 65].rearrange(
                        "p (h c) -> p h c", c=65
                    )[:, :, 0:64]
                    src = pv[:, :384].rearrange("p (h c) -> p h c", c=64)
                    if dh == 0:
                        nc.scalar.copy(dst, src)
                    else:
                        nc.vector.tensor_copy(dst, src)

            gstats_t = bigpool.tile([R, 2], F32)
            nc.sync.dma_start(gstats_t[:], ar_out_d[:])

            # BN coefficients from global stats
            mom_t = bigpool.tile([R, 2], F32)
            nc.vector.tensor_scalar(mom_t[:], gstats_t[:], 1.0 / N_TOT, None, ALU.mult)
            mu_t = mom_t[:, 0:1]
            ex2_t = mom_t[:, 1:2]
            nvar_t = bigpool.tile([R, 1], F32)
            nc.vector.scalar_tensor_tensor(
                nvar_t[:], mu_t[:], mu_t[:], ex2_t[:], ALU.mult, ALU.subtract
            )
            vpe_t = bigpool.tile([R, 1], F32)
            nc.vector.tensor_scalar(vpe_t[:], nvar_t[:], -1.0, 1e-5, ALU.mult, ALU.add)
            sd_t = bigpool.tile([R, 1], F32)
            nc.scalar.sqrt(sd_t[:], vpe_t[:])
            inv_t = bigpool.tile([R, 1], F32)
            nc.vector.reciprocal(inv_t[:], sd_t[:])
            gp_t = bigpool.tile([R, 1], F32)
            nc.vector.tensor_tensor(gp_t[:], gamma_t[:], inv_t[:], ALU.mult)
            mg_t = bigpool.tile([R, 1], F32)
            nc.vector.tensor_tensor(mg_t[:], mu_t[:], gp_t[:], ALU.mult)
            bp_t = bigpool.tile([R, 1], F32)
            nc.vector.tensor_tensor(bp_t[:], beta_t[:], mg_t[:], ALU.subtract)
            nc.vector.tensor_scalar(
                yn_t[:], y_t[:], gp_t[:], bp_t[:], ALU.mult, ALU.add
            )
            yr_t = bigpool.tile([R, 2 * S], F16)
            nc.scalar.activation(yr_t[:], yn_t[:], ACTF.Relu)

            psh = ps1pool.tile([65, S], F32, tag="pacc")
            nc.tensor.matmul(psh[0:H, :], convhT_t[:], yr_t[:, :S], start=True, stop=True)
            sh_sig = bigpool.tile([H, S], F16)
            nc.scalar.activation(sh_sig[:], psh[0:H, :], ACTF.Sigmoid)
            nc.vector.tensor_scalar(sh_t[:], sh_sig[:], 16.0, None, ALU.mult)
            for st in range(KT):
                psw = p2kpool.tile([128, 1024], F32, tag="ps")
                nc.tensor.matmul(
                    psw[:, 0:H],
                    yr_t[:, S + st * 128 : S + (st + 1) * 128],
                    convwT_t[:],
                    start=True,
                    stop=True,
                )
                nc.scalar.activation(sw_cols[:, st, :], psw[:, 0:H], ACTF.Sigmoid)

            # =========== PHASE C: gates + softmax + context ===========
            NB = 1  # output flush batches (DVE base partition must be aligned)
            HB = H // NB
            ctx_stages = []
            for h in range(H):
                # broadcast sh row h across partitions with a one-hot matmul
                pshb = p2kpool.tile([128, 1024], F32, tag="ps")
                nc.tensor.matmul(
                    pshb[:, 0:S],
                    sel_t[:, h * 128 : (h + 1) * 128],
                    sh_t[:],
                    start=True,
                    stop=True,
                )
                shb = shbpool.tile([128, S], F16, tag="shb")
                if h % 2 == 0:
                    nc.scalar.copy(shb[:], pshb[:, 0:S])
                else:
                    nc.vector.tensor_copy(shb[:], pshb[:, 0:S])
                # shbw[kt] = sh[q]*sw[k]; then pre = preps + t*shbw, all in
                # f16 TS(4x)/TT(2x) modes, pre and e written in place over
                # the parked scores
                shbw = prepool.tile([128, KT, S], F16, tag="shbw")
                for kt in range(KT):
                    nc.vector.tensor_scalar(
                        shbw[:, kt, :],
                        shb[:],
                        sw_cols[:, kt, h : h + 1],
                        None,
                        ALU.mult,
                    )
                g_t = epool.tile([128, KT * S], F16, tag="gbuf")
                nc.vector.scalar_tensor_tensor(
                    g_t[:],
                    preps_t[:, h, :],
                    16.0,
                    shbw[:].rearrange("p kt s -> p (kt s)"),
                    ALU.is_ge,
                    ALU.mult,
                )
                nc.vector.tensor_tensor(
                    preps_t[:, h, :], g_t[:], preps_t[:, h, :], ALU.add
                )
                if mask_nonzero:
                    for kt in range(KT):
                        nc.vector.tensor_scalar(
                            preps_t[:, h, kt * S : (kt + 1) * S],
                            preps_t[:, h, kt * S : (kt + 1) * S],
                            mask_t[:, kt : kt + 1],
                            None,
                            ALU.add,
                        )
                nc.scalar.activation(
                    preps_t[:, h, :], preps_t[:, h, :], ACTF.Exp, scale=1.0 / 16.0
                )
                # keepalive: tiny junk MM keyed on this head's gates keeps the
                # PE HAM window warm through the elementwise-heavy stretch
                pjunk = p2kpool.tile([128, 1024], F32, tag="ps")
                nc.tensor.matmul(
                    pjunk[0:1, 0:S],
                    ones_f16[:],
                    g_t[:, 0:S],
                    start=True,
                    stop=True,
                )
                pctx = ps1pool.tile([65, S], F32, tag="pacc")
                for kt in range(KT):
                    nc.tensor.matmul(
                        pctx[:],
                        v_t[:, kt, h * 65 : (h + 1) * 65],
                        preps_t[:, h, kt * S : (kt + 1) * S],
                        start=(kt == 0),
                        stop=(kt == KT - 1),
                    )
                ctx_stage = ctxpool.tile([D + 1, S], F16, tag="ctxs")
                if h % 2 == 0:
                    nc.scalar.copy(ctx_stage[:], pctx[:])
                else:
                    nc.vector.tensor_copy(ctx_stage[:], pctx[:])
                ctx_stages.append(ctx_stage)
                nc.gpsimd.dma_start(
                    den_t[h : h + 1, :], ctx_stage[D : D + 1, :]
                )
                half, hh = divmod(h, HB)
                if hh == HB - 1:
                    hs0 = half * HB
                    hsl = slice(hs0, hs0 + HB)
                    nc.vector.tensor_copy(den32_t[hsl, :], den_t[hsl, :])
                    nc.vector.reciprocal_approx_fast(rec32_t[hsl, :], den32_t[hsl, :])
                    nc.vector.tensor_copy(rec_t[hsl, :], rec32_t[hsl, :])
                    for h2 in range(hs0, hs0 + HB):
                        # broadcast the reciprocal row to D partitions (PE)
                        prb = p2kpool.tile([128, 1024], F32, tag="ps")
                        nc.tensor.matmul(
                            prb[0:D, 0:S],
                            sel_t[:, h2 * 128 : h2 * 128 + D],
                            rec_t[:],
                            start=True,
                            stop=True,
                        )
                        outp = wk3pool.tile([D, S], F16, tag="outp")
                        nc.vector.tensor_tensor(
                            outp[:], ctx_stages[h2][0:D, :], prb[0:D, 0:S], ALU.mult
                        )
                        nc.scalar.dma_start(outT_d[h2 * D : (h2 + 1) * D, :], outp[:])

    nc.compile()
    return nc


def _prep_inputs(
    hidden_states,
    attention_mask,
    Wq,
    bq,
    Wk,
    bk,
    Wv,
    bv,
    conv1_w,
    bn_gamma,
    bn_beta,
    convh_w,
    convw_w,
):
    f32 = np.float32
    f16 = np.float16

    def pad_w(W, b):
        Wp = np.zeros((NI * 128, DM), f32)
        Wp[:DM] = np.asarray(W, f32)
        Wp[DM] = np.asarray(b, f32)
        return Wp

    def col_blocks(Wp):
        # [6(j), 128(p), 7(i), 128(c)]: per-partition contiguous DMA lines
        return np.ascontiguousarray(
            _round_fp32r(Wp).reshape(NI, 128, NT, 128).transpose(2, 1, 0, 3)
        )

    wq_p = col_blocks(pad_w(Wq, bq))
    wk_p = col_blocks(pad_w(Wk, bk))
    wv16_p = np.ascontiguousarray(
        pad_w(Wv, bv).reshape(NI, 128, DM).transpose(1, 0, 2).astype(f16)
    )
    conv1 = np.asarray(conv1_w, f32)
    # counts are scaled to means by folding 1/S into conv1
    conv1T = np.ascontiguousarray((conv1.T / float(S)).astype(f16))
    sel = np.zeros((H, H * 128), f16)
    for h in range(H):
        sel[h, h * 128 : (h + 1) * 128] = 1.0
    convhT = np.ascontiguousarray(np.asarray(convh_w, f32).T.astype(f16))
    convwT = np.ascontiguousarray(np.asarray(convw_w, f32).T.astype(f16))
    gamma = np.asarray(bn_gamma, f32).reshape(R, 1)
    beta = np.asarray(bn_beta, f32).reshape(R, 1)

    hs = np.asarray(hidden_states, f32)
    am = np.asarray(attention_mask, f32)
    in_maps = []
    for b in range(B):
        hsT = np.zeros((NI * 128, S), f32)
        hsT[:DM] = _round_fp32r(hs[b].T)
        hsT[DM] = 1.0
        hsT_p = np.ascontiguousarray(hsT.reshape(NI, 128, S).transpose(1, 0, 2))
        hsT16_p = hsT_p.astype(f16)
        extra = {}
        if np.any(am):
            # 16*mask as per-partition columns [128(k%128), KT(k//128)]
            mc = (16.0 * am[b, 0, 0]).reshape(KT, 128).T
            extra["mask_cols"] = np.ascontiguousarray(mc.astype(f32))
        in_maps.append(
            dict(
                hsT=hsT_p,
                hsT16=hsT16_p,
                wq=wq_p,
                wk=wk_p,
                wv16=wv16_p,
                **extra,
                conv1T=conv1T,
                sel=sel,
                convhT=convhT,
                convwT=convwT,
                gamma=gamma,
                beta=beta,
            )
        )
    return in_maps


def _run(inputs, trace=False, trace_kwargs=None):
    mask_nonzero = bool(np.any(np.asarray(inputs["attention_mask"])))
    bias_nonzero = any(
        bool(np.any(np.asarray(inputs[k]))) for k in ("bq", "bk", "bv")
    )
    key = ("nc", mask_nonzero, bias_nonzero)
    if key not in _CACHE:
        _CACHE[key] = _build(mask_nonzero, bias_nonzero)
    nc = _CACHE[key]
    in_maps = _prep_inputs(**inputs)
    res = run_bass_kernel_spmd(
        nc, in_maps, list(range(8)), trace=trace, **(trace_kwargs or {})
    )
    out = np.stack(
        [np.ascontiguousarray(r["outT"].T).astype(np.float32) for r in res.results]
    )
    return out, res


def kernel(**inputs):
    out, _ = _run(inputs, trace=False)
    return out
